# revision 38
# baseline (speedup 1.0000x reference)
"""DocQA trilinear cross-attention kernel for 8 Trainium2 NeuronCores.

Sharding: data-parallel over batch (B=16 -> 2 batches per core). Params are
tiny and replicated. Each core computes its 2 batches fully; host concatenates.

Per batch b (XL=1024 x-rows, KL=512 key-rows, D=1024), masks are all-ones by
construction (spec fill=ones), so mask terms drop out exactly:
  S[i,j]  = xl[i] + kl[j] + (x[i]*dot_w) . key[j]
  attn    = softmax_j(S)            (xl[i] cancels in softmax_j)
  x2key   = attn @ key
  max_s   = xl + max_j (S - xl)
  p       = softmax_i(max_s)        (the reference's double-normalization
                                     collapses to a single softmax in f32)
  key2x   = p @ x
  out     = concat([x, x2key, x*x2key, x*key2x], -1)

v3 layout strategy: host ships x/key in BOTH natural and transposed layouts
as fp16 (same total bytes as f32 natural, kills all on-device x/key
transposes and casts), outputs chunks 1-3 in fp16 (host upcasts), chunk 0
(x itself) is assembled host-side during the unshard. Engine split: PE does
matmuls + the e-transpose; ACT does exp, keydT scaling and PSUM
evacuation; DVE does xl, reductions, reciprocals and o3; Pool (gpsimd)
alternates with DVE on o4. The key->x chain is kept short (unnormalized
fp16 q = exp(max_s-8); single 1/den scale folded into the k2x evacuation)
and runs on its own PSUM slots so phase B never transitively waits on it.
"""

import json

import numpy as np

import concourse.bass as bass
import concourse.tile as tile
from concourse import masks, mybir

B, XL, KL, D = 16, 1024, 512, 1024
NCORES = 8
BPC = B // NCORES  # batches per core
NIT = XL // 128    # i-tiles per batch
NDC = D // 128     # d chunks (contraction)
NJC = KL // 128    # j chunks

FP = mybir.dt.float32
F16 = mybir.dt.float16
F8 = mybir.dt.float8e4


# --------------------------------------------------------------------------
# BIR post-pass: this container's walrus accepts only ONE sync-wait per
# instruction; Tile emits instructions carrying several. Hoist all but the
# last wait onto standalone single-wait EventSemaphore instructions placed
# immediately before (same engine queue => identical semantics).
# --------------------------------------------------------------------------
_bir_fix_installed = False


def _install_bir_fix():
    global _bir_fix_installed
    if _bir_fix_installed:
        return
    from concourse import bass2jax

    orig_compile = bass2jax.compile_bir_kernel

    def _split_multiwait_compile(bir_bytes, compile_dir, **kw):
        bir = json.loads(bir_bytes)
        n = 0
        for f in bir.get("functions", []):
            for blk in f.get("blocks", []):
                new_insts = []
                for ins in blk.get("instructions", []):
                    si = ins.get("sync_info") or {}
                    waits = si.get("on_wait") or []
                    if len(waits) > 1:
                        for w in waits[:-1]:
                            n += 1
                            new_insts.append({
                                "debug": ins.get("debug", 0),
                                "engine": ins["engine"],
                                "ins": [],
                                "outs": [],
                                "name": f"WSPL-{n}",
                                "opcode": "EventSemaphore",
                                "sync_info": {"on_update": [], "on_wait": [w]},
                            })
                        si["on_wait"] = [waits[-1]]
                    new_insts.append(ins)
                blk["instructions"] = new_insts
        return orig_compile(json.dumps(bir).encode(), compile_dir, **kw)

    bass2jax.compile_bir_kernel = _split_multiwait_compile
    _bir_fix_installed = True


# --------------------------------------------------------------------------
# Kernel program
# --------------------------------------------------------------------------
def build_nc(repeat: int = 1, unroll: bool = False) -> bass.Bass:
    nc = bass.Bass()
    xn_ext = nc.declare_dram_parameter("xn", [BPC, 128, NIT, D], F16, isOutput=False)
    kn_ext = nc.declare_dram_parameter("kn", [BPC, 128, NJC, D], F16, isOutput=False)
    kt_ext = nc.declare_dram_parameter("kt", [BPC, 128, NDC, KL], F16, isOutput=False)
    wib_ext = nc.declare_dram_parameter("wib", [128, D], F16, isOutput=False)
    wk_ext = nc.declare_dram_parameter("wk", [128, NDC], F16, isOutput=False)
    dw_ext = nc.declare_dram_parameter("dw", [128, NDC], FP, isOutput=False)
    o2_ext = nc.declare_dram_parameter("o2", [BPC, XL, D], F16, isOutput=True)
    o3_ext = nc.declare_dram_parameter("o3", [BPC, XL, D], F16, isOutput=True)
    o4_ext = nc.declare_dram_parameter("o4", [BPC, XL, D], F16, isOutput=True)

    with tile.TileContext(nc) as tc:
        from contextlib import ExitStack

        with ExitStack() as ctx:
            ep = ctx.enter_context  # shorthand

            const = ep(tc.tile_pool(name="const", bufs=1))
            inpool = ep(tc.tile_pool(name="inpool", bufs=2))
            kdpool = ep(tc.tile_pool(name="kdpool", bufs=2))
            epool = ep(tc.tile_pool(name="epool", bufs=1))
            work = ep(tc.tile_pool(name="work", bufs=2))
            xtp = ep(tc.tile_pool(name="xtp", bufs=2))
            stage = ep(tc.tile_pool(name="stage", bufs=2))
            small = ep(tc.tile_pool(name="small", bufs=3))
            bpool = ep(tc.tile_pool(name="bpool", bufs=2))

            ps_s = ep(tc.tile_pool(name="ps_s", bufs=2, space="PSUM"))
            ps_x2k = ep(tc.tile_pool(name="ps_x2k", bufs=2, space="PSUM"))
            ps_et = ep(tc.tile_pool(name="ps_et", bufs=2, space="PSUM"))
            ps_misc = ep(tc.tile_pool(name="ps_misc", bufs=1, space="PSUM"))
            ps_tr = ep(tc.tile_pool(name="ps_tr", bufs=1, space="PSUM"))

            # ---- constants ----
            ident = const.tile([128, 128], F16, tag="ident")
            masks.make_identity(nc, ident[:])
            ones_row = const.tile([1, 128], F16, tag="ones_row")
            nc.gpsimd.memset(ones_row[:], 1.0)
            ones_col = const.tile([128, 1], FP, tag="ones_col")
            nc.gpsimd.memset(ones_col[:], 1.0)
            ones_row_f = const.tile([1, 128], FP, tag="ones_row_f")
            nc.gpsimd.memset(ones_row_f[:], 1.0)
            wib_sb = const.tile([128, D], F16, tag="wib")
            nc.sync.dma_start(wib_sb[:], wib_ext[:])
            wk_sb = const.tile([128, NDC], F16, tag="wk")
            nc.sync.dma_start(wk_sb[:], wk_ext[:])
            dw_sb = const.tile([128, NDC], FP, tag="dw")
            nc.sync.dma_start(dw_sb[:], dw_ext[:])
            qbias = const.tile([128, 1], FP, tag="qbias")
            nc.gpsimd.memset(qbias[:], -8.0)

            def body():
                def emit_batch_loads(b):
                    t = {}
                    kt = inpool.tile([128, NDC, KL], F16, tag="kt", name=f"kt{b}")
                    nc.sync.dma_start(kt[:], kt_ext[b])
                    t["kt"] = kt
                    xn = inpool.tile([128, NIT, D], F16, tag="xn", name=f"xn{b}")
                    nc.sync.dma_start(xn[:], xn_ext[b])
                    t["xn"] = xn
                    kn = inpool.tile([128, NJC, D], F16, tag="kn", name=f"kn{b}")
                    nc.sync.dma_start(kn[:], kn_ext[b])
                    t["kn"] = kn
                    return t

                tiles = emit_batch_loads(0)
                for b in range(BPC):
                    cur = tiles
                    xn, kn, kt = cur["xn"], cur["kn"], cur["kt"]

                    # ====== per-batch key prep ======
                    # keydT[c] = dot_w[c-chunk] * keyT[c]  (fp16)
                    kdt = kdpool.tile([128, NDC, KL], F16, tag="kdt")
                    for c in range(NDC):
                        nc.scalar.activation(
                            kdt[:, c, :], kt[:, c, :],
                            mybir.ActivationFunctionType.Copy,
                            scale=dw_sb[:, c:c + 1],
                        )
                    # kl[j] = w_key . key[j]  (row layout via wk-stationary MMs)
                    klp = ps_misc.tile([1, KL], FP, tag="b_ps")
                    for c in range(NDC):
                        nc.tensor.matmul(
                            klp[:], wk_sb[:, c:c + 1], kt[:, c, :],
                            start=(c == 0), stop=(c == NDC - 1),
                        )
                    kl_row = small.tile([1, KL], F16, tag="kl_row", bufs=2)
                    nc.scalar.activation(
                        kl_row[:], klp[:], mybir.ActivationFunctionType.Copy
                    )

                    nm_all = bpool.tile([128, NIT], FP, tag="nm_all")
                    es_all = bpool.tile([128, NIT], FP, tag="es_all")
                    xl_all = bpool.tile([128, NIT], FP, tag="xl_all")
                    xl_scr = bpool.tile([128, D], F16, tag="xl_scr")
                    e_tiles = []

                    # ====== phase A: scores, row-max, exp, xl ======
                    # on-device x transpose: T(it) -> evac(it) -> S(it),
                    # emitted as T0, [evac0, T1, S0], [evac1, T2, S1], ...
                    def emit_xtr(it):
                        trp = ps_tr.tile([128, D], F16, tag="tr_ps",
                                         name=f"trp{b}_{it}")
                        for c in range(NDC):
                            nc.tensor.transpose(
                                trp[:, c * 128:(c + 1) * 128],
                                xn[:, it, c * 128:(c + 1) * 128], ident[:],
                            )
                        return trp

                    def emit_xtr_evac(trp, it):
                        xt_it = xtp.tile([128, D], F16, tag="xt_it",
                                         name=f"xt{b}_{it}")
                        nc.vector.tensor_copy(xt_it[:], trp[:])
                        return xt_it

                    trp = emit_xtr(0)
                    xt_its = [emit_xtr_evac(trp, 0)]
                    for it in range(NIT):
                        if it + 1 < NIT:
                            trp = emit_xtr(it + 1)
                            xt_its.append(emit_xtr_evac(trp, it + 1))
                        xt_it = xt_its[it]
                        # xl column on DVE: xl[i] = sum_d x[i,d]*wi[d]
                        nc.vector.scalar_tensor_tensor(
                            xl_scr[:], xn[:, it, :], 1.0, wib_sb[:],
                            op0=mybir.AluOpType.mult, op1=mybir.AluOpType.mult,
                            accum_out=xl_all[:, it:it + 1],
                        )

                        # S = kl (bcast) + (x*dw) . key^T
                        sp = ps_s.tile([128, NJC, 128], FP, tag="s_ps")
                        nc.tensor.matmul(sp[:], ones_row[:], kl_row[:],
                                         start=True, stop=False)
                        for c in range(NDC):
                            nc.tensor.matmul(
                                sp[:], xt_it[:, c * 128:(c + 1) * 128],
                                kdt[:, c, :],
                                start=False, stop=(c == NDC - 1),
                            )

                        # row max (negated) -> nm column
                        nc.vector.tensor_reduce(
                            nm_all[:, it:it + 1], sp[:],
                            axis=mybir.AxisListType.XY,
                            op=mybir.AluOpType.max, negate=True,
                        )

                        # e = exp(S) kept for phase B; row sums in es_all
                        e_sb = epool.tile([128, NJC, 128], F16, tag=f"e_{it}")
                        nc.scalar.activation(
                            e_sb[:], sp[:],
                            mybir.ActivationFunctionType.Exp,
                            accum_out=es_all[:, it:it + 1],
                        )
                        e_tiles.append(e_sb)

                    # hoist next batch loads ahead of this batch's stores
                    if b + 1 < BPC:
                        tiles = emit_batch_loads(b + 1)

                    # ====== key -> x attention ======
                    # q = exp(max_s - 8) unnormalized (fits fp16); the 1/den
                    # normalization happens once in the k2x evacuation scale.
                    max_s = small.tile([128, NIT], FP, tag="max_s")
                    nc.vector.tensor_sub(max_s[:], xl_all[:], nm_all[:])
                    pnum = small.tile([128, NIT], F16, tag="pnum")
                    nc.scalar.activation(
                        pnum[:], max_s[:], mybir.ActivationFunctionType.Exp,
                        bias=qbias[:],
                    )
                    den = small.tile([1, 1], FP, tag="den")
                    nc.gpsimd.tensor_reduce(
                        den[:], pnum[:], axis=mybir.AxisListType.XYZWC,
                        op=mybir.AluOpType.add,
                    )
                    rden = small.tile([1, 1], FP, tag="rden")
                    nc.vector.reciprocal(rden[:], den[:])

                    # key2x = (q @ x) / den  (fp16 matmuls on xn tiles)
                    k2x = small.tile([1, D], F16, tag="k2x", bufs=2)
                    for h in range(2):
                        kxp = ps_misc.tile([1, 512], FP, tag="b_ps")
                        for it in range(NIT):
                            nc.tensor.matmul(
                                kxp[:], pnum[:, it:it + 1],
                                xn[:, it, h * 512:(h + 1) * 512],
                                start=(it == 0), stop=(it == NIT - 1),
                            )
                        nc.scalar.activation(
                            k2x[:, h * 512:(h + 1) * 512], kxp[:],
                            mybir.ActivationFunctionType.Copy, scale=rden[:],
                        )
                    # broadcast key2x to all partitions on PE + ACT copies
                    k2b = bpool.tile([128, D], F16, tag="k2b")
                    for h in range(2):
                        kbp = ps_misc.tile([128, 512], FP, tag="b_ps")
                        nc.tensor.matmul(
                            kbp[:], ones_row[:], k2x[0:1, h * 512:(h + 1) * 512],
                            start=True, stop=True,
                        )
                        nc.scalar.activation(
                            k2b[:, h * 512:(h + 1) * 512], kbp[:],
                            mybir.ActivationFunctionType.Copy,
                        )

                    # ====== phase B: attention outputs ======
                    for it in range(NIT):
                        e_sb = e_tiles[it]
                        rs = small.tile([128, 1], FP, tag="rs")
                        nc.vector.reciprocal(rs[:], es_all[:, it:it + 1])

                        etp = ps_et.tile([128, NJC, 128], F16, tag="et_ps")
                        for jc in range(NJC):
                            nc.tensor.transpose(
                                etp[:, jc, :], e_sb[:, jc, :], ident[:],
                            )
                        et = work.tile([128, NJC, 128], F16, tag="et_sb")
                        nc.scalar.activation(
                            et[:], etp[:], mybir.ActivationFunctionType.Copy
                        )

                        x2k = stage.tile([128, D], F16, tag="x2k")
                        for h in range(2):
                            xkp = ps_x2k.tile([128, 512], FP, tag="x2k_ps")
                            for jc in range(NJC):
                                nc.tensor.matmul(
                                    xkp[:], et[:, jc, :],
                                    kn[:, jc, h * 512:(h + 1) * 512],
                                    start=(jc == 0), stop=(jc == NJC - 1),
                                )
                            nc.scalar.activation(
                                x2k[:, h * 512:(h + 1) * 512], xkp[:],
                                mybir.ActivationFunctionType.Copy, scale=rs[:],
                            )

                        r0, r1 = it * 128, (it + 1) * 128
                        nc.sync.dma_start(o2_ext[b, r0:r1, :], x2k[:])
                        o3 = stage.tile([128, D], F16, tag="o3")
                        nc.vector.tensor_mul(o3[:], xn[:, it, :], x2k[:])
                        nc.sync.dma_start(o3_ext[b, r0:r1, :], o3[:])
                        o4 = stage.tile([128, D], F16, tag="o4")
                        if it % 2 == 0:
                            nc.gpsimd.tensor_tensor(
                                o4[:], xn[:, it, :], k2b[:],
                                op=mybir.AluOpType.mult,
                            )
                        else:
                            nc.vector.tensor_mul(o4[:], xn[:, it, :], k2b[:])
                        nc.sync.dma_start(o4_ext[b, r0:r1, :], o4[:])

            if repeat == 1:
                body()
            elif unroll:
                for _ in range(repeat):
                    body()
            else:
                with tc.For_i(0, repeat, 1):
                    body()

    return nc


# --------------------------------------------------------------------------
# Host entry point
# --------------------------------------------------------------------------
_cache = {}


def _get_nc(repeat: int = 1) -> bass.Bass:
    if repeat not in _cache:
        _cache[repeat] = build_nc(repeat)
    return _cache[repeat]


def make_in_maps(x, x_mask, key, key_mask, w_input, w_key, dot_w):
    x16 = np.asarray(x, np.float32).astype(np.float16)
    k16 = np.asarray(key, np.float32).astype(np.float16)
    wib = np.ascontiguousarray(
        np.broadcast_to(np.asarray(w_input, np.float32).astype(np.float16), (128, D))
    )
    wk = np.ascontiguousarray(
        np.asarray(w_key, np.float32).reshape(NDC, 128).T
    ).astype(np.float16)
    dw = np.ascontiguousarray(np.asarray(dot_w, np.float32).reshape(NDC, 128).T)
    in_maps = []
    for c in range(NCORES):
        s = slice(c * BPC, (c + 1) * BPC)
        xn = np.ascontiguousarray(
            x16[s].reshape(BPC, NIT, 128, D).transpose(0, 2, 1, 3))
        kn = np.ascontiguousarray(
            k16[s].reshape(BPC, NJC, 128, D).transpose(0, 2, 1, 3))
        kt = np.ascontiguousarray(
            k16[s].reshape(BPC, KL, NDC, 128).transpose(0, 3, 2, 1))
        in_maps.append({
            "xn": xn, "kn": kn, "kt": kt,
            "wib": wib, "wk": wk, "dw": dw,
        })
    return in_maps


def kernel(x, x_mask, key, key_mask, w_input, w_key, dot_w):
    from concourse.bass_utils import run_bass_kernel_spmd

    _install_bir_fix()
    nc = _get_nc(1)
    in_maps = make_in_maps(x, x_mask, key, key_mask, w_input, w_key, dot_w)
    res = run_bass_kernel_spmd(nc, in_maps, list(range(NCORES)))
    x = np.asarray(x, np.float32)
    out = np.empty((B, XL, 4 * D), np.float32)
    out[..., 0:D] = x
    for c in range(NCORES):
        s = slice(c * BPC, (c + 1) * BPC)
        r = res.results[c]
        out[s, :, D:2 * D] = r["o2"].astype(np.float32)
        out[s, :, 2 * D:3 * D] = r["o3"].astype(np.float32)
        out[s, :, 3 * D:4 * D] = r["o4"].astype(np.float32)
    return out


# revision 41
# speedup vs baseline: 1.1116x; 1.1116x over previous
"""DocQA trilinear cross-attention kernel for 8 Trainium2 NeuronCores.

Sharding: data-parallel over batch (B=16 -> 2 batches per core). Params are
tiny and replicated. Each core computes its 2 batches fully; host concatenates.

Per batch b (XL=1024 x-rows, KL=512 key-rows, D=1024), masks are all-ones by
construction (spec fill=ones), so mask terms drop out exactly:
  S[i,j]  = xl[i] + kl[j] + (x[i]*dot_w) . key[j]
  attn    = softmax_j(S)            (xl[i] cancels in softmax_j)
  x2key   = attn @ key
  max_s   = xl + max_j (S - xl)
  p       = softmax_i(max_s)        (the reference's double-normalization
                                     collapses to a single softmax in f32)
  key2x   = p @ x
  out     = concat([x, x2key, x*x2key, x*key2x], -1)

v3 layout strategy: host ships x/key in BOTH natural and transposed layouts
as fp16 (same total bytes as f32 natural, kills all on-device x/key
transposes and casts), outputs chunks 1-3 in fp16 (host upcasts), chunk 0
(x itself) is assembled host-side during the unshard. Engine split: PE does
matmuls + the e-transpose; ACT does exp, keydT scaling and PSUM
evacuation; DVE does xl, reductions, reciprocals and o3; Pool (gpsimd)
alternates with DVE on o4. The key->x chain is kept short (unnormalized
fp16 q = exp(max_s-8); single 1/den scale folded into the k2x evacuation)
and runs on its own PSUM slots so phase B never transitively waits on it.
"""

import json

import numpy as np

import concourse.bass as bass
import concourse.tile as tile
from concourse import masks, mybir

B, XL, KL, D = 16, 1024, 512, 1024
NCORES = 8
BPC = B // NCORES  # batches per core
NIT = XL // 128    # i-tiles per batch
NDC = D // 128     # d chunks (contraction)
NJC = KL // 128    # j chunks

FP = mybir.dt.float32
F16 = mybir.dt.float16
F8 = mybir.dt.float8e4


# --------------------------------------------------------------------------
# BIR post-pass: this container's walrus accepts only ONE sync-wait per
# instruction; Tile emits instructions carrying several. Hoist all but the
# last wait onto standalone single-wait EventSemaphore instructions placed
# immediately before (same engine queue => identical semantics).
# --------------------------------------------------------------------------
_bir_fix_installed = False


def _install_bir_fix():
    global _bir_fix_installed
    if _bir_fix_installed:
        return
    from concourse import bass2jax

    orig_compile = bass2jax.compile_bir_kernel

    def _split_multiwait_compile(bir_bytes, compile_dir, **kw):
        bir = json.loads(bir_bytes)
        n = 0
        for f in bir.get("functions", []):
            for blk in f.get("blocks", []):
                new_insts = []
                for ins in blk.get("instructions", []):
                    si = ins.get("sync_info") or {}
                    waits = si.get("on_wait") or []
                    if len(waits) > 1:
                        for w in waits[:-1]:
                            n += 1
                            new_insts.append({
                                "debug": ins.get("debug", 0),
                                "engine": ins["engine"],
                                "ins": [],
                                "outs": [],
                                "name": f"WSPL-{n}",
                                "opcode": "EventSemaphore",
                                "sync_info": {"on_update": [], "on_wait": [w]},
                            })
                        si["on_wait"] = [waits[-1]]
                    new_insts.append(ins)
                blk["instructions"] = new_insts
        return orig_compile(json.dumps(bir).encode(), compile_dir, **kw)

    bass2jax.compile_bir_kernel = _split_multiwait_compile
    _bir_fix_installed = True


# --------------------------------------------------------------------------
# Kernel program
# --------------------------------------------------------------------------
def build_nc(repeat: int = 1, unroll: bool = False) -> bass.Bass:
    nc = bass.Bass()
    xn_ext = nc.declare_dram_parameter("xn", [BPC, 128, NIT, D], F16, isOutput=False)
    xt_ext = nc.declare_dram_parameter("xt", [BPC, 128, NDC, XL], F16, isOutput=False)
    kn_ext = nc.declare_dram_parameter("kn", [BPC, 128, NJC, D], F16, isOutput=False)
    kt_ext = nc.declare_dram_parameter("kt", [BPC, 128, NDC, KL], F16, isOutput=False)
    wib_ext = nc.declare_dram_parameter("wib", [128, D], F16, isOutput=False)
    wk_ext = nc.declare_dram_parameter("wk", [128, NDC], F16, isOutput=False)
    dw_ext = nc.declare_dram_parameter("dw", [128, NDC], FP, isOutput=False)
    o2_ext = nc.declare_dram_parameter("o2", [BPC, XL, D], F16, isOutput=True)
    o3_ext = nc.declare_dram_parameter("o3", [BPC, XL, D], F16, isOutput=True)
    o4_ext = nc.declare_dram_parameter("o4", [BPC, XL, D], F16, isOutput=True)

    with tile.TileContext(nc) as tc:
        from contextlib import ExitStack

        with ExitStack() as ctx:
            ep = ctx.enter_context  # shorthand

            const = ep(tc.tile_pool(name="const", bufs=1))
            inpool = ep(tc.tile_pool(name="inpool", bufs=2))
            kdpool = ep(tc.tile_pool(name="kdpool", bufs=2))
            epool = ep(tc.tile_pool(name="epool", bufs=2))
            work = ep(tc.tile_pool(name="work", bufs=2))
            stage = ep(tc.tile_pool(name="stage", bufs=2))
            small = ep(tc.tile_pool(name="small", bufs=3))
            bpool = ep(tc.tile_pool(name="bpool", bufs=2))

            ps_s = ep(tc.tile_pool(name="ps_s", bufs=2, space="PSUM"))
            ps_x2k = ep(tc.tile_pool(name="ps_x2k", bufs=2, space="PSUM"))
            ps_et = ep(tc.tile_pool(name="ps_et", bufs=2, space="PSUM"))
            ps_misc = ep(tc.tile_pool(name="ps_misc", bufs=2, space="PSUM"))

            # ---- constants ----
            ident = const.tile([128, 128], F16, tag="ident")
            masks.make_identity(nc, ident[:])
            ones_row = const.tile([1, 128], F16, tag="ones_row")
            nc.gpsimd.memset(ones_row[:], 1.0)
            ones_col = const.tile([128, 1], FP, tag="ones_col")
            nc.gpsimd.memset(ones_col[:], 1.0)
            ones_row_f = const.tile([1, 128], FP, tag="ones_row_f")
            nc.gpsimd.memset(ones_row_f[:], 1.0)
            wib_sb = const.tile([128, D], F16, tag="wib")
            nc.sync.dma_start(wib_sb[:], wib_ext[:])
            wk_sb = const.tile([128, NDC], F16, tag="wk")
            nc.sync.dma_start(wk_sb[:], wk_ext[:])
            dw_sb = const.tile([128, NDC], FP, tag="dw")
            nc.sync.dma_start(dw_sb[:], dw_ext[:])
            qbias = const.tile([128, 1], FP, tag="qbias")
            nc.gpsimd.memset(qbias[:], -8.0)

            def body():
                def emit_batch_loads(b):
                    t = {}
                    kt = inpool.tile([128, NDC, KL], F16, tag="kt", name=f"kt{b}")
                    nc.sync.dma_start(kt[:], kt_ext[b])
                    t["kt"] = kt
                    xt = inpool.tile([128, NDC, XL], F16, tag="xt", name=f"xt{b}")
                    nc.sync.dma_start(xt[:], xt_ext[b])
                    t["xt"] = xt
                    xn = inpool.tile([128, NIT, D], F16, tag="xn", name=f"xn{b}")
                    nc.sync.dma_start(xn[:], xn_ext[b])
                    t["xn"] = xn
                    kn = inpool.tile([128, NJC, D], F16, tag="kn", name=f"kn{b}")
                    nc.sync.dma_start(kn[:], kn_ext[b])
                    t["kn"] = kn
                    return t

                all_tiles = [emit_batch_loads(0), emit_batch_loads(1)]
                state = []
                for b in range(BPC):
                    cur = all_tiles[b]
                    xn, xt, kn, kt = cur["xn"], cur["xt"], cur["kn"], cur["kt"]

                    # ====== per-batch key prep ======
                    # keydT[c] = dot_w[c-chunk] * keyT[c]  (fp16)
                    kdt = kdpool.tile([128, NDC, KL], F16, tag="kdt")
                    for c in range(NDC):
                        nc.vector.tensor_scalar(
                            kdt[:, c, :], kt[:, c, :], dw_sb[:, c:c + 1], None,
                            op0=mybir.AluOpType.mult,
                        )
                    # kl[j] = w_key . key[j]  (row layout via wk-stationary MMs)
                    klp = ps_misc.tile([1, KL], FP, tag="b_ps")
                    for c in range(NDC):
                        nc.tensor.matmul(
                            klp[:], wk_sb[:, c:c + 1], kt[:, c, :],
                            start=(c == 0), stop=(c == NDC - 1),
                        )
                    kl_row = small.tile([1, KL], F16, tag="kl_row", bufs=2)
                    nc.scalar.activation(
                        kl_row[:], klp[:], mybir.ActivationFunctionType.Copy
                    )

                    nm_all = bpool.tile([128, NIT], FP, tag="nm_all")
                    es_all = bpool.tile([128, NIT], FP, tag="es_all")
                    xl_all = bpool.tile([128, NIT], FP, tag="xl_all")
                    xl_scr = bpool.tile([128, D], F16, tag="xl_scr")
                    e_tiles = []

                    # ====== phase A: scores, row-max, exp, xl ======
                    for it in range(NIT):
                        # xl column on DVE: xl[i] = sum_d x[i,d]*wi[d]
                        nc.vector.scalar_tensor_tensor(
                            xl_scr[:], xn[:, it, :], 1.0, wib_sb[:],
                            op0=mybir.AluOpType.mult, op1=mybir.AluOpType.mult,
                            accum_out=xl_all[:, it:it + 1],
                        )

                        # S = kl (bcast) + (x*dw) . key^T
                        sp = ps_s.tile([128, NJC, 128], FP, tag="s_ps")
                        nc.tensor.matmul(sp[:], ones_row[:], kl_row[:],
                                         start=True, stop=False)
                        for c in range(NDC):
                            nc.tensor.matmul(
                                sp[:], xt[:, c, it * 128:(it + 1) * 128],
                                kdt[:, c, :],
                                start=False, stop=(c == NDC - 1),
                            )

                        # row max (negated) -> nm column
                        nc.vector.tensor_reduce(
                            nm_all[:, it:it + 1], sp[:],
                            axis=mybir.AxisListType.XY,
                            op=mybir.AluOpType.max, negate=True,
                        )

                        # e = exp(S) kept for phase B; row sums in es_all
                        e_sb = epool.tile([128, NJC, 128], F16, tag=f"e_{it}")
                        nc.scalar.activation(
                            e_sb[:], sp[:],
                            mybir.ActivationFunctionType.Exp,
                            accum_out=es_all[:, it:it + 1],
                        )
                        e_tiles.append(e_sb)

                    state.append(dict(
                        kl_row=kl_row, nm_all=nm_all, es_all=es_all,
                        xl_all=xl_all, e_tiles=e_tiles))

                for b in range(BPC):
                    cur = all_tiles[b]
                    xn, xt, kn, kt = cur["xn"], cur["xt"], cur["kn"], cur["kt"]
                    st = state[b]
                    nm_all, es_all = st["nm_all"], st["es_all"]
                    xl_all, e_tiles = st["xl_all"], st["e_tiles"]

                    # ====== key -> x attention ======
                    # q = exp(max_s - 8) unnormalized (fits fp16); the 1/den
                    # normalization happens once in the k2x evacuation scale.
                    max_s = small.tile([128, NIT], FP, tag="max_s")
                    nc.vector.tensor_sub(max_s[:], xl_all[:], nm_all[:])
                    pnum = small.tile([128, NIT], F16, tag="pnum")
                    nc.scalar.activation(
                        pnum[:], max_s[:], mybir.ActivationFunctionType.Exp,
                        bias=qbias[:],
                    )
                    den = small.tile([1, 1], FP, tag="den")
                    nc.gpsimd.tensor_reduce(
                        den[:], pnum[:], axis=mybir.AxisListType.XYZWC,
                        op=mybir.AluOpType.add,
                    )
                    rden = small.tile([1, 1], FP, tag="rden")
                    nc.vector.reciprocal(rden[:], den[:])

                    # key2x = (q @ x) / den  (fp16 matmuls on xn tiles)
                    k2x = small.tile([1, D], F16, tag="k2x", bufs=2)
                    for h in range(2):
                        kxp = ps_misc.tile([1, 512], FP, tag="b_ps")
                        for it in range(NIT):
                            nc.tensor.matmul(
                                kxp[:], pnum[:, it:it + 1],
                                xn[:, it, h * 512:(h + 1) * 512],
                                start=(it == 0), stop=(it == NIT - 1),
                            )
                        nc.scalar.activation(
                            k2x[:, h * 512:(h + 1) * 512], kxp[:],
                            mybir.ActivationFunctionType.Copy, scale=rden[:],
                        )
                    # broadcast key2x to all partitions on PE + ACT copies
                    k2b = bpool.tile([128, D], F16, tag="k2b")
                    for h in range(2):
                        kbp = ps_misc.tile([128, 512], FP, tag="b_ps")
                        nc.tensor.matmul(
                            kbp[:], ones_row[:], k2x[0:1, h * 512:(h + 1) * 512],
                            start=True, stop=True,
                        )
                        nc.scalar.activation(
                            k2b[:, h * 512:(h + 1) * 512], kbp[:],
                            mybir.ActivationFunctionType.Copy,
                        )

                    # ====== phase B: attention outputs ======
                    for it in range(NIT):
                        e_sb = e_tiles[it]
                        rs = small.tile([128, 1], FP, tag="rs")
                        nc.vector.reciprocal(rs[:], es_all[:, it:it + 1])

                        etp = ps_et.tile([128, NJC, 128], F16, tag="et_ps")
                        for jc in range(NJC):
                            nc.tensor.transpose(
                                etp[:, jc, :], e_sb[:, jc, :], ident[:],
                            )
                        et = work.tile([128, NJC, 128], F16, tag="et_sb")
                        nc.scalar.activation(
                            et[:], etp[:], mybir.ActivationFunctionType.Copy
                        )

                        x2k = stage.tile([128, D], F16, tag="x2k")
                        for h in range(2):
                            xkp = ps_x2k.tile([128, 512], FP, tag="x2k_ps")
                            for jc in range(NJC):
                                nc.tensor.matmul(
                                    xkp[:], et[:, jc, :],
                                    kn[:, jc, h * 512:(h + 1) * 512],
                                    start=(jc == 0), stop=(jc == NJC - 1),
                                )
                            nc.scalar.activation(
                                x2k[:, h * 512:(h + 1) * 512], xkp[:],
                                mybir.ActivationFunctionType.Copy, scale=rs[:],
                            )

                        r0, r1 = it * 128, (it + 1) * 128
                        nc.sync.dma_start(o2_ext[b, r0:r1, :], x2k[:])
                        o3 = stage.tile([128, D], F16, tag="o3")
                        nc.vector.tensor_mul(o3[:], xn[:, it, :], x2k[:])
                        nc.sync.dma_start(o3_ext[b, r0:r1, :], o3[:])
                        o4 = stage.tile([128, D], F16, tag="o4")
                        if it % 2 == 0:
                            nc.gpsimd.tensor_tensor(
                                o4[:], xn[:, it, :], k2b[:],
                                op=mybir.AluOpType.mult,
                            )
                        else:
                            nc.vector.tensor_mul(o4[:], xn[:, it, :], k2b[:])
                        nc.sync.dma_start(o4_ext[b, r0:r1, :], o4[:])

            if repeat == 1:
                body()
            elif unroll:
                for _ in range(repeat):
                    body()
            else:
                with tc.For_i(0, repeat, 1):
                    body()

    return nc


# --------------------------------------------------------------------------
# Host entry point
# --------------------------------------------------------------------------
_cache = {}


def _get_nc(repeat: int = 1) -> bass.Bass:
    if repeat not in _cache:
        _cache[repeat] = build_nc(repeat)
    return _cache[repeat]


def make_in_maps(x, x_mask, key, key_mask, w_input, w_key, dot_w):
    x16 = np.asarray(x, np.float32).astype(np.float16)
    k16 = np.asarray(key, np.float32).astype(np.float16)
    wib = np.ascontiguousarray(
        np.broadcast_to(np.asarray(w_input, np.float32).astype(np.float16), (128, D))
    )
    wk = np.ascontiguousarray(
        np.asarray(w_key, np.float32).reshape(NDC, 128).T
    ).astype(np.float16)
    dw = np.ascontiguousarray(np.asarray(dot_w, np.float32).reshape(NDC, 128).T)
    in_maps = []
    for c in range(NCORES):
        s = slice(c * BPC, (c + 1) * BPC)
        xn = np.ascontiguousarray(
            x16[s].reshape(BPC, NIT, 128, D).transpose(0, 2, 1, 3))
        xt = np.ascontiguousarray(
            x16[s].reshape(BPC, XL, NDC, 128).transpose(0, 3, 2, 1))
        kn = np.ascontiguousarray(
            k16[s].reshape(BPC, NJC, 128, D).transpose(0, 2, 1, 3))
        kt = np.ascontiguousarray(
            k16[s].reshape(BPC, KL, NDC, 128).transpose(0, 3, 2, 1))
        in_maps.append({
            "xn": xn, "xt": xt, "kn": kn, "kt": kt,
            "wib": wib, "wk": wk, "dw": dw,
        })
    return in_maps


def kernel(x, x_mask, key, key_mask, w_input, w_key, dot_w):
    from concourse.bass_utils import run_bass_kernel_spmd

    _install_bir_fix()
    nc = _get_nc(1)
    in_maps = make_in_maps(x, x_mask, key, key_mask, w_input, w_key, dot_w)
    res = run_bass_kernel_spmd(nc, in_maps, list(range(NCORES)))
    x = np.asarray(x, np.float32)
    out = np.empty((B, XL, 4 * D), np.float32)
    out[..., 0:D] = x
    for c in range(NCORES):
        s = slice(c * BPC, (c + 1) * BPC)
        r = res.results[c]
        out[s, :, D:2 * D] = r["o2"].astype(np.float32)
        out[s, :, 2 * D:3 * D] = r["o3"].astype(np.float32)
        out[s, :, 3 * D:4 * D] = r["o4"].astype(np.float32)
    return out
